# revision 1
# baseline (speedup 1.0000x reference)
"""Swin-style window attention kernel for Trainium2 (8 NeuronCores, data-parallel).

Computes, for x:[2048,49,384]:
    qkv = x @ qkv_w.T + qkv_b ; split into q,k,v heads (12 x 32)
    attn = softmax(q k^T / sqrt(32) + rel_pos_bias + window_mask)
    out  = (attn @ v) @ proj_w.T

Strategy: data-parallel over the leading B_ axis (256 windows / core).
On-chip layout is channel-major (x pre-transposed on host), windows are
processed in pairs (98 tokens) so attention matmuls use 98-wide tiles.
Relative-position bias + window mask are folded into one multiplicative
term EB = exp(bias + mask) precomputed on the host; softmax is computed
without max-subtraction (scores are O(1) here) as exp(s)*EB / colsum.
All matmuls run in bf16 with fp32 PSUM accumulation.
"""

import sys

sys.path.insert(0, "/opt/trn_rl_repo")

import numpy as np
import ml_dtypes

import concourse.bacc as bacc
import concourse.mybir as mybir
import concourse.tile as tile
from concourse.bass_utils import run_bass_kernel_spmd

BF16 = ml_dtypes.bfloat16
F32 = np.float32

N_CORES = 8
D, H, HD = 384, 12, 32
WN = 49                      # tokens per window
NW = 64                      # distinct window masks
B_ = 2048
B_CORE = B_ // N_CORES       # 256 windows per core
T_CORE = B_CORE * WN         # 12544 tokens per core
PT = 2 * WN                  # 98 tokens per window pair
N_PAIR = B_CORE // 2         # 128 pairs per core
PAIR_PAT = NW // 2           # 32 distinct pair mask patterns
BLK_PAIRS = 8
BLK_T = BLK_PAIRS * PT       # 784 tokens per block
N_BLK = N_PAIR // BLK_PAIRS  # 16 blocks per core
NH = BLK_T // 2              # 392: half-block free dim for 512-limit psum
SCALE = HD ** (-0.5)

_BF = mybir.dt.bfloat16
_F32 = mybir.dt.float32


def _relative_position_index():
    coords = np.stack(np.meshgrid(np.arange(7), np.arange(7), indexing="ij"))
    cf = coords.reshape(2, -1)
    rel = cf[:, :, None] - cf[:, None, :]
    rel = rel.transpose(1, 2, 0).copy()
    rel[:, :, 0] += 6
    rel[:, :, 1] += 6
    rel[:, :, 0] *= 13
    return rel.sum(-1)  # [49, 49] int


def _build_nc(qkv_bias_nonzero: bool, reps: int = 1, skip: frozenset = frozenset()):
    nc = bacc.Bacc("TRN2", target_bir_lowering=False, debug=True)

    xT_d = nc.dram_tensor("xT", [D, T_CORE], _BF, kind="ExternalInput")
    wqk_d = nc.dram_tensor("wqk", [128, 3, 2 * D], _BF, kind="ExternalInput")
    wv_d = nc.dram_tensor("wv", [128, 3, D], _BF, kind="ExternalInput")
    pw_d = nc.dram_tensor("pw", [128, 3, D], _BF, kind="ExternalInput")
    eb_d = nc.dram_tensor("eb", [PT, PAIR_PAT, H, PT], _BF, kind="ExternalInput")
    bqk_d = nc.dram_tensor("bqk", [1, 2 * D], _BF, kind="ExternalInput")
    bv_d = nc.dram_tensor("bv", [1, D], _BF, kind="ExternalInput")
    yT_d = nc.dram_tensor("yT", [D, T_CORE], _BF, kind="ExternalOutput")

    xT_view = xT_d[:, :].rearrange("(k p) t -> p k t", p=128)
    yT_view = yT_d[:, :].rearrange("(k p) t -> p k t", p=128)

    with tile.TileContext(nc) as tc:
        with (
            tc.tile_pool(name="consts", bufs=1) as consts,
            tc.tile_pool(name="xin", bufs=2) as xin,
            tc.tile_pool(name="qkp", bufs=2) as qkp,
            tc.tile_pool(name="vp", bufs=2) as vp,
            tc.tile_pool(name="attnp", bufs=4) as attnp,
            tc.tile_pool(name="rp", bufs=2) as rp,
            tc.tile_pool(name="outp", bufs=2) as outp,
            tc.tile_pool(name="yp", bufs=2) as yp,
            tc.tile_pool(name="ps_mm", bufs=2, space="PSUM") as ps_mm,
            tc.tile_pool(name="ps_s", bufs=1, space="PSUM") as ps_s,
            tc.tile_pool(name="ps_cs", bufs=1, space="PSUM") as ps_cs,
            tc.tile_pool(name="ps_o", bufs=1, space="PSUM") as ps_o,
        ):
            # ---- constants ----
            wqk_sb = consts.tile([128, 3, 2 * D], _BF)
            nc.sync.dma_start(out=wqk_sb, in_=wqk_d[:, :, :])
            wv_sb = consts.tile([128, 3, D], _BF)
            nc.sync.dma_start(out=wv_sb, in_=wv_d[:, :, :])
            pw_sb = consts.tile([128, 3, D], _BF)
            nc.sync.dma_start(out=pw_sb, in_=pw_d[:, :, :])
            eb_sb = consts.tile([PT, PAIR_PAT, H, PT], _BF)
            nc.sync.dma_start(out=eb_sb, in_=eb_d[:, :, :, :])
            ones_sb = consts.tile([PT, 32], _BF)
            nc.vector.memset(ones_sb, 1.0)
            if "scores" in skip:
                attn0_sb = consts.tile([PT, H, PT], _BF, name="attn0")
                nc.gpsimd.memset(attn0_sb, 0.5)
            if "av" in skip:
                outN0_sb = consts.tile([128, 3, BLK_PAIRS, PT], _BF, name="outN0")
                nc.gpsimd.memset(outN0_sb, 0.5)
            if qkv_bias_nonzero:
                bqk_sb = consts.tile([1, 2 * D], _BF)
                nc.sync.dma_start(out=bqk_sb, in_=bqk_d[:, :])
                bv_sb = consts.tile([1, D], _BF)
                nc.sync.dma_start(out=bv_sb, in_=bv_d[:, :])
                onetok_sb = consts.tile([1, NH], _BF)
                nc.vector.memset(onetok_sb, 1.0)

            for blk_rep in range(N_BLK * reps):
                blk = blk_rep % N_BLK
                t0 = blk * BLK_T
                # ---- load x^T block ----
                xT_sb = xin.tile([128, 3, BLK_T], _BF)
                nc.sync.dma_start(out=xT_sb, in_=xT_view[:, :, t0 : t0 + BLK_T])

                # ---- qk^T projection: channel-major [768, BLK_T] ----
                qk_sb = qkp.tile([128, 6, BLK_T], _BF)
                for nh in range(2):
                    for m in range(6):
                        mm_ps = ps_mm.tile([128, NH], _F32, tag="mm")
                        for k in range(3):
                            nc.tensor.matmul(
                                out=mm_ps,
                                lhsT=wqk_sb[:, k, 128 * m : 128 * (m + 1)],
                                rhs=xT_sb[:, k, nh * NH : (nh + 1) * NH],
                                start=(k == 0),
                                stop=(k == 2) if not qkv_bias_nonzero else False,
                            )
                        if qkv_bias_nonzero:
                            nc.tensor.matmul(
                                out=mm_ps,
                                lhsT=bqk_sb[:, 128 * m : 128 * (m + 1)],
                                rhs=onetok_sb,
                                start=False,
                                stop=True,
                            )
                        nc.scalar.copy(
                            out=qk_sb[:, m, nh * NH : (nh + 1) * NH], in_=mm_ps
                        )

                # ---- v projection: token-major [98, 384] per pair ----
                v_sb = vp.tile([PT, BLK_PAIRS, D], _BF)
                for p8 in range(BLK_PAIRS):
                    v_ps = ps_mm.tile([PT, D], _F32, tag="mm", name="v_ps")
                    for k in range(3):
                        nc.tensor.matmul(
                            out=v_ps,
                            lhsT=xT_sb[:, k, p8 * PT : (p8 + 1) * PT],
                            rhs=wv_sb[:, k, :],
                            start=(k == 0),
                            stop=(k == 2) if not qkv_bias_nonzero else False,
                        )
                    if qkv_bias_nonzero:
                        nc.tensor.matmul(
                            out=v_ps,
                            lhsT=onetok_sb[:, :PT],
                            rhs=bv_sb,
                            start=False,
                            stop=True,
                        )
                    if p8 % 2 == 0:
                        nc.scalar.copy(out=v_sb[:, p8, :], in_=v_ps)
                    else:
                        nc.vector.tensor_copy(out=v_sb[:, p8, :], in_=v_ps)

                outN_sb = (
                    outN0_sb if "av" in skip
                    else outp.tile([128, 3, BLK_PAIRS, PT], _BF)
                )

                for p8 in range(BLK_PAIRS):
                    pair = blk * BLK_PAIRS + p8
                    pr = pair % PAIR_PAT
                    ts = p8 * PT

                    # ---- scores: s k_h^T q_h, h = 4g + j ----
                    # bank j <- row-group j (heads j, 4+j, 8+j): same-bank
                    # matmuls share a row-group (serialize), concurrent
                    # row-groups write different banks.
                    if "scores" in skip:
                        attn_sb = attn0_sb
                    else:
                        # one 4-bank PSUM tile; bank j <- row-group j only
                        # (regions r = 4j+g at 512B offsets, 4 regions/bank),
                        # so same-bank matmuls share a row-group (serialize)
                        # while concurrent row-groups hit different banks.
                        s_ps = ps_s.tile([PT, 16, 128], _F32, tag="s", name="s4")
                        for r in range(3):
                            for j in range(4):
                                g = (j + r) % 3
                                nc.tensor.matmul(
                                    out=s_ps[:, 4 * j + g, :PT],
                                    lhsT=qk_sb[
                                        32 * j : 32 * (j + 1), 3 + g, ts : ts + PT
                                    ],
                                    rhs=qk_sb[32 * j : 32 * (j + 1), g, ts : ts + PT],
                                    start=True,
                                    stop=True,
                                    tile_position=(32 * j, 0),
                                )

                        # ---- attn = exp(s) * EB, one op each ----
                        # attn head order is (j, g): idx = 3j + g = head 4g+j
                        attn_sb = attnp.tile([PT, H, PT], _BF)
                        s_v = s_ps.rearrange("p (j g) c -> p j g c", g=4)
                        for j in range(4):
                            nc.scalar.activation(
                                out=attn_sb[:, 3 * j : 3 * (j + 1), :],
                                in_=s_v[:, j, :3, :PT],
                                func=mybir.ActivationFunctionType.Exp,
                            )
                        if "ebmul" not in skip:
                            nc.vector.tensor_mul(
                                attn_sb, attn_sb, eb_sb[:, pr, :, :]
                            )

                    if "av" in skip:
                        continue
                    # ---- softmax denominators: column sums via PE ----
                    # all-ones [98,32] stationary operand -> each head's
                    # colsum lands replicated across its 32 output rows.
                    cs_ps = ps_cs.tile([128, 3, PT], _F32, tag="cs")
                    for r in range(4):
                        for g in range(3):
                            j = (g + r) % 4
                            nc.tensor.matmul(
                                out=cs_ps[32 * j : 32 * (j + 1), g, :],
                                lhsT=ones_sb,
                                rhs=attn_sb[:, 3 * j + g, :],
                                start=True,
                                stop=True,
                                tile_position=(0, 32 * j),
                            )
                    r_sb = rp.tile([128, 3, PT], _BF, tag="rsb")
                    with nc.allow_low_precision(reason="bf16 softmax recip"):
                        nc.vector.reciprocal(out=r_sb, in_=cs_ps)

                    # ---- attn @ v (unnormalized), col-tiled by head ----
                    o_ps = ps_o.tile([128, 3, PT], _F32, tag="o")
                    for r in range(4):
                        for g in range(3):
                            j = (g + r) % 4
                            h = 4 * g + j
                            nc.tensor.matmul(
                                out=o_ps[32 * j : 32 * (j + 1), g, :],
                                lhsT=v_sb[:, p8, 32 * h : 32 * (h + 1)],
                                rhs=attn_sb[:, 3 * j + g, :],
                                start=True,
                                stop=True,
                                tile_position=(0, 32 * j),
                            )

                    # ---- normalize: outN = o * recip (one op) ----
                    nc.vector.tensor_mul(
                        outN_sb[:, :, p8, :], o_ps[:, :, :], r_sb[:, :, :]
                    )

                # ---- proj: yT = pw^T.T @ outN ----
                yT_sb = yp.tile([128, 3, BLK_T], _BF)
                for nh in range(2):
                    for m in range(3):
                        y_ps = ps_mm.tile([128, NH], _F32, tag="mm")
                        for k in range(3):
                            nc.tensor.matmul(
                                out=y_ps,
                                lhsT=pw_sb[:, k, 128 * m : 128 * (m + 1)],
                                rhs=outN_sb[:, k, nh * 4 : (nh + 1) * 4, :],
                                start=(k == 0),
                                stop=(k == 2),
                            )
                        nc.scalar.copy(
                            out=yT_sb[:, m, nh * NH : (nh + 1) * NH], in_=y_ps
                        )
                nc.sync.dma_start(out=yT_view[:, :, t0 : t0 + BLK_T], in_=yT_sb)

    nc.compile()
    return nc


_NC_CACHE: dict = {}


def _get_nc(qkv_bias_nonzero: bool):
    key = qkv_bias_nonzero
    if key not in _NC_CACHE:
        _NC_CACHE[key] = _build_nc(qkv_bias_nonzero)
    return _NC_CACHE[key]


def _host_prep(x, mask, qkv_w, qkv_b, proj_w, rpb_table):
    """Build per-core input maps (all device tensors bf16)."""
    # x^T per core: [384, 12544], channel-major
    x8 = np.ascontiguousarray(x, dtype=F32).reshape(N_CORES, T_CORE, D)

    # weights: lhsT layout [ci, co] chunked as [128, 3, co]
    wqkv_t = np.ascontiguousarray(qkv_w, dtype=F32).T  # [384, 1152] = [ci, co]
    wqk = wqkv_t[:, : 2 * D].copy()
    wqk[:, :D] *= SCALE  # fold 1/sqrt(hd) into q weights
    wv = wqkv_t[:, 2 * D :]
    pw_t = np.ascontiguousarray(proj_w, dtype=F32).T  # [ci, co]

    def chunk(w):  # [384, co] -> [128, 3, co]
        return np.ascontiguousarray(
            w.reshape(3, 128, w.shape[1]).transpose(1, 0, 2)
        ).astype(BF16)

    wqk_a, wv_a, pw_a = chunk(wqk), chunk(wv), chunk(pw_t)

    # EB = exp(biasT + maskT) per pair pattern, [98, 32, 12, 98] (j, pr, h, i)
    rpi = _relative_position_index()
    bias = np.asarray(rpb_table, dtype=F32)[rpi]          # [i, j, H]
    biasT = bias.transpose(2, 1, 0)                        # [H, j, i]
    maskT = np.asarray(mask, dtype=F32).transpose(0, 2, 1)  # [w, j, i]
    mb = np.full((PAIR_PAT, H, PT, PT), -30000.0, dtype=F32)
    mb[:, :, :WN, :WN] = biasT[None] + maskT[0::2, None, :, :]
    mb[:, :, WN:, WN:] = biasT[None] + maskT[1::2, None, :, :]
    eb = np.exp(mb)
    # device head order is (j, g): idx = 3j + g holds head h = 4g + j
    perm = np.array([4 * (i % 3) + i // 3 for i in range(H)])
    eb = eb[:, perm]
    eb_a = np.ascontiguousarray(eb.transpose(2, 0, 1, 3)).astype(BF16)

    b = np.asarray(qkv_b, dtype=F32)
    bqk = b[: 2 * D].copy()
    bqk[:D] *= SCALE
    bqk_a = bqk[None, :].astype(BF16)
    bv_a = b[2 * D :][None, :].astype(BF16)

    in_maps = []
    for c in range(N_CORES):
        xT_c = np.ascontiguousarray(x8[c].T).astype(BF16)  # [384, 12544]
        in_maps.append(
            {
                "xT": xT_c,
                "wqk": wqk_a,
                "wv": wv_a,
                "pw": pw_a,
                "eb": eb_a,
                "bqk": bqk_a,
                "bv": bv_a,
            }
        )
    return in_maps


def kernel(x, mask, qkv_w, qkv_b, proj_w, rpb_table, _want_trace=False):
    qkv_bias_nonzero = bool(np.any(np.asarray(qkv_b) != 0))
    nc = _get_nc(qkv_bias_nonzero)
    in_maps = _host_prep(x, mask, qkv_w, qkv_b, proj_w, rpb_table)
    res = run_bass_kernel_spmd(
        nc, in_maps, core_ids=list(range(N_CORES)), trace=_want_trace
    )
    yT = np.stack([res.results[c]["yT"].astype(F32) for c in range(N_CORES)])
    y = yT.transpose(0, 2, 1).reshape(B_, WN, D)
    if _want_trace:
        kernel._last_result = res
    return y



# revision 9
# speedup vs baseline: 1.2924x; 1.2924x over previous
"""Swin-style window attention kernel for Trainium2 (8 NeuronCores, data-parallel).

Computes, for x:[2048,49,384]:
    qkv = x @ qkv_w.T + qkv_b ; split into q,k,v heads (12 x 32)
    attn = softmax(q k^T / sqrt(32) + rel_pos_bias + window_mask)
    out  = (attn @ v) @ proj_w.T

Strategy: data-parallel over the leading B_ axis (256 windows / core).
On-chip layout is channel-major (x pre-transposed on host), windows are
processed in pairs (98 tokens) so attention matmuls use 98-wide tiles.
Relative-position bias + window mask are folded into one multiplicative
term EB = exp(bias + mask) precomputed on the host; softmax is computed
without max-subtraction (scores are O(1) here) as exp(s)*EB / colsum.
All matmuls run in bf16 with fp32 PSUM accumulation.
"""

import sys

sys.path.insert(0, "/opt/trn_rl_repo")

import numpy as np
import ml_dtypes

import concourse.bacc as bacc
import concourse.mybir as mybir
import concourse.tile as tile
from concourse.bass_utils import run_bass_kernel_spmd

BF16 = ml_dtypes.bfloat16
F32 = np.float32

N_CORES = 8
D, H, HD = 384, 12, 32
WN = 49                      # tokens per window
NW = 64                      # distinct window masks
B_ = 2048
B_CORE = B_ // N_CORES       # 256 windows per core
T_CORE = B_CORE * WN         # 12544 tokens per core
PT = 2 * WN                  # 98 tokens per window pair
N_PAIR = B_CORE // 2         # 128 pairs per core
PAIR_PAT = NW // 2           # 32 distinct pair mask patterns
BLK_PAIRS = 8
BLK_T = BLK_PAIRS * PT       # 784 tokens per block
N_BLK = N_PAIR // BLK_PAIRS  # 16 blocks per core
NH = BLK_T // 2              # 392: half-block free dim for 512-limit psum
SCALE = HD ** (-0.5)

_BF = mybir.dt.bfloat16
_F32 = mybir.dt.float32


def _relative_position_index():
    coords = np.stack(np.meshgrid(np.arange(7), np.arange(7), indexing="ij"))
    cf = coords.reshape(2, -1)
    rel = cf[:, :, None] - cf[:, None, :]
    rel = rel.transpose(1, 2, 0).copy()
    rel[:, :, 0] += 6
    rel[:, :, 1] += 6
    rel[:, :, 0] *= 13
    return rel.sum(-1)  # [49, 49] int


def _build_nc(qkv_bias_nonzero: bool, reps: int = 1, skip: frozenset = frozenset()):
    nc = bacc.Bacc("TRN2", target_bir_lowering=False, debug=True)

    xT_d = nc.dram_tensor("xT", [D, T_CORE], _BF, kind="ExternalInput")
    wqk_d = nc.dram_tensor("wqk", [128, 3, 2 * D], _BF, kind="ExternalInput")
    wv_d = nc.dram_tensor("wv", [128, 3, D], _BF, kind="ExternalInput")
    pw_d = nc.dram_tensor("pw", [128, 3, D], _BF, kind="ExternalInput")
    eb_d = nc.dram_tensor("eb", [PT, PAIR_PAT, H, PT], _BF, kind="ExternalInput")
    bqk_d = nc.dram_tensor("bqk", [1, 2 * D], _BF, kind="ExternalInput")
    bv_d = nc.dram_tensor("bv", [1, D], _BF, kind="ExternalInput")
    yT_d = nc.dram_tensor("yT", [D, T_CORE], _BF, kind="ExternalOutput")

    xT_view = xT_d[:, :].rearrange("(k p) t -> p k t", p=128)
    yT_view = yT_d[:, :].rearrange("(k p) t -> p k t", p=128)

    with tile.TileContext(nc) as tc:
        with (
            tc.tile_pool(name="consts", bufs=1) as consts,
            tc.tile_pool(name="xin", bufs=2) as xin,
            tc.tile_pool(name="qkp", bufs=2) as qkp,
            tc.tile_pool(name="vp", bufs=2) as vp,
            tc.tile_pool(name="attnp", bufs=4) as attnp,
            tc.tile_pool(name="rp", bufs=2) as rp,
            tc.tile_pool(name="outp", bufs=2) as outp,
            tc.tile_pool(name="yp", bufs=2) as yp,
            tc.tile_pool(name="ps_mm", bufs=2, space="PSUM") as ps_mm,
            tc.tile_pool(name="ps_s", bufs=1, space="PSUM") as ps_s,
            tc.tile_pool(name="ps_cs", bufs=1, space="PSUM") as ps_cs,
            tc.tile_pool(name="ps_o", bufs=1, space="PSUM") as ps_o,
        ):
            # ---- constants ----
            wqk_sb = consts.tile([128, 3, 2 * D], _BF)
            nc.sync.dma_start(out=wqk_sb, in_=wqk_d[:, :, :])
            wv_sb = consts.tile([128, 3, D], _BF)
            nc.sync.dma_start(out=wv_sb, in_=wv_d[:, :, :])
            pw_sb = consts.tile([128, 3, D], _BF)
            nc.sync.dma_start(out=pw_sb, in_=pw_d[:, :, :])
            eb_sb = consts.tile([PT, PAIR_PAT, H, PT], _BF)
            nc.sync.dma_start(out=eb_sb, in_=eb_d[:, :, :, :])
            ones_sb = consts.tile([PT, 32], _BF)
            nc.vector.memset(ones_sb, 1.0)
            if "scores" in skip:
                attn0_sb = consts.tile([PT, H, PT], _BF, name="attn0")
                nc.gpsimd.memset(attn0_sb, 0.5)
            if "av" in skip:
                outN0_sb = consts.tile([128, 3, BLK_PAIRS, PT], _BF, name="outN0")
                nc.gpsimd.memset(outN0_sb, 0.5)
            if qkv_bias_nonzero:
                bqk_sb = consts.tile([1, 2 * D], _BF)
                nc.sync.dma_start(out=bqk_sb, in_=bqk_d[:, :])
                bv_sb = consts.tile([1, D], _BF)
                nc.sync.dma_start(out=bv_sb, in_=bv_d[:, :])
                onetok_sb = consts.tile([1, NH], _BF)
                nc.vector.memset(onetok_sb, 1.0)

            for blk_rep in range(N_BLK * reps):
                blk = blk_rep % N_BLK
                t0 = blk * BLK_T
                # ---- load x^T block ----
                xT_sb = xin.tile([128, 3, BLK_T], _BF)
                nc.sync.dma_start(out=xT_sb, in_=xT_view[:, :, t0 : t0 + BLK_T])

                # ---- qk^T projection: channel-major [768, BLK_T] ----
                qk_sb = qkp.tile([128, 6, BLK_T], _BF)
                for nh in range(2):
                    for m in range(6):
                        mm_ps = ps_mm.tile([128, NH], _F32, tag="mm")
                        for k in range(3):
                            nc.tensor.matmul(
                                out=mm_ps,
                                lhsT=wqk_sb[:, k, 128 * m : 128 * (m + 1)],
                                rhs=xT_sb[:, k, nh * NH : (nh + 1) * NH],
                                start=(k == 0),
                                stop=(k == 2) if not qkv_bias_nonzero else False,
                            )
                        if qkv_bias_nonzero:
                            nc.tensor.matmul(
                                out=mm_ps,
                                lhsT=bqk_sb[:, 128 * m : 128 * (m + 1)],
                                rhs=onetok_sb,
                                start=False,
                                stop=True,
                            )
                        if m % 2 == 0:
                            nc.scalar.copy(
                                out=qk_sb[:, m, nh * NH : (nh + 1) * NH], in_=mm_ps
                            )
                        else:
                            nc.vector.tensor_copy(
                                out=qk_sb[:, m, nh * NH : (nh + 1) * NH], in_=mm_ps
                            )

                # ---- v projection: token-major [98, 384] per pair ----
                v_sb = vp.tile([PT, BLK_PAIRS, D], _BF)
                for p8 in range(BLK_PAIRS):
                    v_ps = ps_mm.tile([PT, D], _F32, tag="mm", name="v_ps")
                    for k in range(3):
                        nc.tensor.matmul(
                            out=v_ps,
                            lhsT=xT_sb[:, k, p8 * PT : (p8 + 1) * PT],
                            rhs=wv_sb[:, k, :],
                            start=(k == 0),
                            stop=(k == 2) if not qkv_bias_nonzero else False,
                        )
                    if qkv_bias_nonzero:
                        nc.tensor.matmul(
                            out=v_ps,
                            lhsT=onetok_sb[:, :PT],
                            rhs=bv_sb,
                            start=False,
                            stop=True,
                        )
                    if p8 % 2 == 0:
                        nc.scalar.copy(out=v_sb[:, p8, :], in_=v_ps)
                    else:
                        nc.vector.tensor_copy(out=v_sb[:, p8, :], in_=v_ps)

                outN_sb = (
                    outN0_sb if "av" in skip
                    else outp.tile([128, 3, BLK_PAIRS, PT], _BF)
                )

                for p8 in range(BLK_PAIRS):
                    pair = blk * BLK_PAIRS + p8
                    pr = pair % PAIR_PAT
                    ts = p8 * PT

                    # ---- scores: s k_h^T q_h, h = 4g + j ----
                    # bank j <- row-group j (heads j, 4+j, 8+j): same-bank
                    # matmuls share a row-group (serialize), concurrent
                    # row-groups write different banks.
                    if "scores" in skip:
                        attn_sb = attn0_sb
                    else:
                        # one 4-bank PSUM tile; bank j <- row-group j only
                        # (regions r = 4j+g at 512B offsets, 4 regions/bank),
                        # so same-bank matmuls share a row-group (serialize)
                        # while concurrent row-groups hit different banks.
                        s_ps = ps_s.tile([PT, 16, 128], _F32, tag="s", name="s4")
                        for r in range(3):
                            for j in range(4):
                                g = (j + r) % 3
                                nc.tensor.matmul(
                                    out=s_ps[:, 4 * j + g, :PT],
                                    lhsT=qk_sb[
                                        32 * j : 32 * (j + 1), 3 + g, ts : ts + PT
                                    ],
                                    rhs=qk_sb[32 * j : 32 * (j + 1), g, ts : ts + PT],
                                    start=True,
                                    stop=True,
                                    tile_position=(32 * j, 0),
                                )

                        # ---- attn = exp(s) * EB, one EXP per PSUM bank ----
                        # attn head order is (j, g): idx = 3j + g = head 4g+j
                        attn_sb = attnp.tile([PT, H, PT], _BF)
                        s_v = s_ps.rearrange("p (j g) c -> p j g c", g=4)
                        for j in range(4):
                            nc.scalar.activation(
                                out=attn_sb[:, 3 * j : 3 * (j + 1), :],
                                in_=s_v[:, j, :3, :PT],
                                func=mybir.ActivationFunctionType.Exp,
                            )
                        if "ebmul" not in skip:
                            nc.vector.tensor_mul(
                                attn_sb, attn_sb, eb_sb[:, pr, :, :]
                            )

                    if "av" in skip:
                        continue
                    # ---- softmax denominators: column sums via PE ----
                    # all-ones [98,32] stationary operand -> each head's
                    # colsum lands replicated across its 32 output rows.
                    # One matmul per row-group j covers its 3 heads at once.
                    cs_ps = ps_cs.tile([128, 3, PT], _F32, tag="cs")
                    for j in range(4):
                        nc.tensor.matmul(
                            out=cs_ps[32 * j : 32 * (j + 1), :, :],
                            lhsT=ones_sb,
                            rhs=attn_sb[:, 3 * j : 3 * (j + 1), :],
                            start=True,
                            stop=True,
                            tile_position=(0, 32 * j),
                        )
                    r_sb = rp.tile([128, 3, PT], _F32, tag="rsb")
                    nc.vector.reciprocal_approx_fast(out=r_sb, in_=cs_ps)

                    # ---- attn @ v (unnormalized), col-tiled by head ----
                    o_ps = ps_o.tile([128, 3, PT], _F32, tag="o")
                    for r in range(4):
                        for g in range(3):
                            j = (g + r) % 4
                            h = 4 * g + j
                            nc.tensor.matmul(
                                out=o_ps[32 * j : 32 * (j + 1), g, :],
                                lhsT=v_sb[:, p8, 32 * h : 32 * (h + 1)],
                                rhs=attn_sb[:, 3 * j + g, :],
                                start=True,
                                stop=True,
                                tile_position=(0, 32 * j),
                            )

                    # ---- normalize: outN = o * recip (one op) ----
                    nc.vector.tensor_mul(
                        outN_sb[:, :, p8, :], o_ps[:, :, :], r_sb[:, :, :]
                    )

                # ---- proj: yT = pw^T.T @ outN ----
                yT_sb = yp.tile([128, 3, BLK_T], _BF)
                for nh in range(2):
                    for m in range(3):
                        y_ps = ps_mm.tile([128, NH], _F32, tag="mm")
                        for k in range(3):
                            nc.tensor.matmul(
                                out=y_ps,
                                lhsT=pw_sb[:, k, 128 * m : 128 * (m + 1)],
                                rhs=outN_sb[:, k, nh * 4 : (nh + 1) * 4, :],
                                start=(k == 0),
                                stop=(k == 2),
                            )
                        if m % 2 == 0:
                            nc.scalar.copy(
                                out=yT_sb[:, m, nh * NH : (nh + 1) * NH], in_=y_ps
                            )
                        else:
                            nc.vector.tensor_copy(
                                out=yT_sb[:, m, nh * NH : (nh + 1) * NH], in_=y_ps
                            )
                nc.sync.dma_start(out=yT_view[:, :, t0 : t0 + BLK_T], in_=yT_sb)

    nc.compile()
    return nc


_NC_CACHE: dict = {}


def _get_nc(qkv_bias_nonzero: bool):
    key = qkv_bias_nonzero
    if key not in _NC_CACHE:
        _NC_CACHE[key] = _build_nc(qkv_bias_nonzero)
    return _NC_CACHE[key]


def _host_prep(x, mask, qkv_w, qkv_b, proj_w, rpb_table):
    """Build per-core input maps (all device tensors bf16)."""
    # x^T per core: [384, 12544], channel-major
    x8 = np.ascontiguousarray(x, dtype=F32).reshape(N_CORES, T_CORE, D)

    # weights: lhsT layout [ci, co] chunked as [128, 3, co]
    wqkv_t = np.ascontiguousarray(qkv_w, dtype=F32).T  # [384, 1152] = [ci, co]
    wqk = wqkv_t[:, : 2 * D].copy()
    wqk[:, :D] *= SCALE  # fold 1/sqrt(hd) into q weights
    wv = wqkv_t[:, 2 * D :]
    pw_t = np.ascontiguousarray(proj_w, dtype=F32).T  # [ci, co]

    def chunk(w):  # [384, co] -> [128, 3, co]
        return np.ascontiguousarray(
            w.reshape(3, 128, w.shape[1]).transpose(1, 0, 2)
        ).astype(BF16)

    wqk_a, wv_a, pw_a = chunk(wqk), chunk(wv), chunk(pw_t)

    # EB = exp(biasT + maskT) per pair pattern, [98, 32, 12, 98] (j, pr, h, i)
    rpi = _relative_position_index()
    bias = np.asarray(rpb_table, dtype=F32)[rpi]          # [i, j, H]
    biasT = bias.transpose(2, 1, 0)                        # [H, j, i]
    maskT = np.asarray(mask, dtype=F32).transpose(0, 2, 1)  # [w, j, i]
    mb = np.full((PAIR_PAT, H, PT, PT), -30000.0, dtype=F32)
    mb[:, :, :WN, :WN] = biasT[None] + maskT[0::2, None, :, :]
    mb[:, :, WN:, WN:] = biasT[None] + maskT[1::2, None, :, :]
    eb = np.exp(mb)
    # device head order is (j, g): idx = 3j + g holds head h = 4g + j
    perm = np.array([4 * (i % 3) + i // 3 for i in range(H)])
    eb = eb[:, perm]
    eb_a = np.ascontiguousarray(eb.transpose(2, 0, 1, 3)).astype(BF16)

    b = np.asarray(qkv_b, dtype=F32)
    bqk = b[: 2 * D].copy()
    bqk[:D] *= SCALE
    bqk_a = bqk[None, :].astype(BF16)
    bv_a = b[2 * D :][None, :].astype(BF16)

    in_maps = []
    for c in range(N_CORES):
        xT_c = np.ascontiguousarray(x8[c].T).astype(BF16)  # [384, 12544]
        in_maps.append(
            {
                "xT": xT_c,
                "wqk": wqk_a,
                "wv": wv_a,
                "pw": pw_a,
                "eb": eb_a,
                "bqk": bqk_a,
                "bv": bv_a,
            }
        )
    return in_maps


def kernel(x, mask, qkv_w, qkv_b, proj_w, rpb_table, _want_trace=False):
    qkv_bias_nonzero = bool(np.any(np.asarray(qkv_b) != 0))
    nc = _get_nc(qkv_bias_nonzero)
    in_maps = _host_prep(x, mask, qkv_w, qkv_b, proj_w, rpb_table)
    res = run_bass_kernel_spmd(
        nc, in_maps, core_ids=list(range(N_CORES)), trace=_want_trace
    )
    yT = np.stack([res.results[c]["yT"].astype(F32) for c in range(N_CORES)])
    y = yT.transpose(0, 2, 1).reshape(B_, WN, D)
    if _want_trace:
        kernel._last_result = res
    return y

